# revision 11
# baseline (speedup 1.0000x reference)
"""Trainium2 Bass kernel for a single-head attention block (B=8, S=2048, D=512, dk=dv=64).

Sharding: one batch element per NeuronCore (8 cores, data parallel).

Per-core algorithm (batch b), all in "transposed" layouts chosen so that every
matmul contraction runs over the SBUF partition axis:

  host:   qT/kT/vT = q[b].T etc                           [D=512, S=2048]
  proj:   qpT[d,s] = sum_D Wq[D,d] qT[D,s] + bq[d]        [64, S]   (kpT same)
          vp[t,dv] = sum_D vT[D,t] Wv[D,dv] + bv[dv]      [S, 64]   (natural)
  scores: sT[t,s]  = sum_d kpT[d,t] qpT[d,s]              (t chunks of 128)
  P       = exp(sT * 1/8 + pad[t])  (ACT, scale+bias fused; no max-subtraction:
            scores are O(5), exp is safe in fp32 and matches the reference
            softmax exactly up to fp rounding)
  causal  = multiply P by 0/1 upper-tri mask on diagonal tiles; skip tiles
            that are fully masked
  AV:     avT[dv,s] = sum_t vpe[t,dv] P[t,s]  with vpe = [vp * E | E],
            E[t] = exp(pad[t])  (folds the pad mask into vp);  row 64 of avT
            is then the softmax denominator sum_t P~[t,s]
  out:    out[s,dv] = avT[dv,s] / (avT[64,s] + 1e-10)  (PE transpose + DVE)
"""

import numpy as np

B, S, D, DK, DV = 8, 2048, 512, 64, 64
NCORES = 8
SC = 512              # s-chunk (attention column) width
NSC = S // SC         # 4
TCH = 128             # t-chunk height
NT = S // TCH         # 16

CFG = dict(
    mm_dtype="float32r",   # "float32r" (fast, TF32-like) or "float32" (exact)
    trace=False,           # collect NTFF profile (set by test.py)
)

_prog = None


def _build_program():
    from contextlib import ExitStack

    import concourse.bass as bass
    import concourse.mybir as mybir
    import concourse.tile as tile
    from concourse import bacc

    f32 = mybir.dt.float32
    # np view of float32r is float32, so host arrays are unchanged; engines
    # writing an f32r tile round it so the BIR verifier accepts matmul reads
    mmdt = getattr(mybir.dt, CFG["mm_dtype"])

    nc = bacc.Bacc(
        trn_type="TRN2",
        target_bir_lowering=False,
        debug=False,
        num_devices=NCORES,
    )

    # qkT[cc] = [qT[64cc:64cc+64, :]; kT[64cc:64cc+64, :]] — 64-deep D-chunks of
    # q and k interleaved so one DMA feeds the block-diagonal projection matmul
    qkT = nc.dram_tensor("qkT", [8, 128, S], mmdt, kind="ExternalInput").ap()
    vT = nc.dram_tensor("vT", [D, S], f32, kind="ExternalInput").ap()
    # wqk[cc] = [[Wq[64cc:64cc+64, :], 0], [0, Wk[64cc:64cc+64, :]]] (block-diag)
    wqk_d = nc.dram_tensor("wqk", [128, 8 * 128], mmdt, kind="ExternalInput").ap()
    # wv packed: [p, (c, m)] with Wv[128c + p, m] at [p, 64c + m]
    wv_d = nc.dram_tensor("wv", [128, 256], f32, kind="ExternalInput").ap()
    bias_d = nc.dram_tensor("bias", [64, 3], f32, kind="ExternalInput").ap()
    bvrow_d = nc.dram_tensor("bvrow", [1, DV], f32, kind="ExternalInput").ap()
    padT_d = nc.dram_tensor("padT", [128, NT], f32, kind="ExternalInput").ap()
    out_d = nc.dram_tensor("out", [S, DV], f32, kind="ExternalOutput").ap()

    with tile.TileContext(nc) as tc:
        with ExitStack() as ctx:
            const = ctx.enter_context(tc.tile_pool(name="const", bufs=1))
            xin = ctx.enter_context(tc.tile_pool(name="xin", bufs=8))
            pp = ctx.enter_context(tc.tile_pool(name="pp", bufs=3))
            sbw = ctx.enter_context(tc.tile_pool(name="sbw", bufs=3))
            ps_qk = ctx.enter_context(tc.tile_pool(name="ps_qk", bufs=2, space="PSUM"))
            ps_pj = ctx.enter_context(tc.tile_pool(name="ps_pj", bufs=2, space="PSUM"))
            ps_pjv = ctx.enter_context(tc.tile_pool(name="ps_pjv", bufs=1, space="PSUM"))
            ps_av = ctx.enter_context(tc.tile_pool(name="ps_av", bufs=1, space="PSUM"))

            # ---- constants ----
            wqk = const.tile([128, 8 * 128], mmdt, tag="wqk")
            nc.sync.dma_start(out=wqk[:], in_=wqk_d[:])
            wv = const.tile([128, 256], f32, tag="wv")
            nc.sync.dma_start(out=wv[:], in_=wv_d[:])
            bias = const.tile([64, 3], f32, tag="bias")
            nc.sync.dma_start(out=bias[:], in_=bias_d[:])
            padT = const.tile([128, NT], f32, tag="padT")
            nc.sync.dma_start(out=padT[:], in_=padT_d[:])
            # bv broadcast across partitions (bv varies along the free axis of vp)
            bvb = const.tile([128, DV], f32, tag="bvb")
            nc.sync.dma_start(out=bvb[:], in_=bvrow_d.partition_broadcast(128))

            # E[t] = exp(pad[t])
            E = const.tile([128, NT], f32, tag="E")
            nc.scalar.activation(E[:], padT[:], mybir.ActivationFunctionType.Exp)

            # identity for PE transposes
            ident = const.tile([128, 128], f32, tag="ident")
            nc.gpsimd.memset(ident[:], 0.0)
            nc.gpsimd.affine_select(
                out=ident[:],
                in_=ident[:],
                compare_op=mybir.AluOpType.not_equal,
                fill=1.0,
                base=0,
                pattern=[[-1, 128]],
                channel_multiplier=1,
            )

            # shifted causal mask bank (additive): maskbig[u, x] = 0 if
            # x >= u + 384 (allowed) else -1e30 (masked). Slice for diagonal
            # tile r: mask_r[u, w] = maskbig[u, w + 384 - 128*r]
            maskbig = const.tile([128, 896], f32, tag="maskbig")
            nc.gpsimd.memset(maskbig[:], 0.0)
            nc.gpsimd.affine_select(
                out=maskbig[:],
                in_=maskbig[:],
                compare_op=mybir.AluOpType.is_ge,
                fill=-1.0e30,
                base=-384,
                pattern=[[1, 896]],
                channel_multiplier=-1,
            )

            # per-s-chunk projection outputs (SBUF resident)
            qpT = [
                const.tile([64, SC], mmdt, tag=f"qpT{i}", name=f"qpT{i}")
                for i in range(NSC)
            ]
            kpT = [
                const.tile([64, SC], mmdt, tag=f"kpT{i}", name=f"kpT{i}")
                for i in range(NSC)
            ]
            # per-t-chunk scaled V projections [vp * E | E]
            vpe = [
                const.tile([128, DV + 1], mmdt, tag=f"vpe{j}", name=f"vpe{j}")
                for j in range(NT)
            ]

            Exp = mybir.ActivationFunctionType.Exp

            for sc in range(NSC):
                ssl = bass.ts(sc, SC)  # DRAM s-range of this chunk

                # ---- projections for s-chunk sc ----
                pj = ps_pj.tile([128, SC], f32, tag="pj")
                pjv = ps_pjv.tile([128, 4 * DV], f32, tag="pjv")
                # q & k projections via block-diagonal weights: out rows 0:64
                # = qpT chunk, rows 64:128 = kpT chunk, one matmul per 64-deep
                # D-chunk (full 128-wide array, single accumulation group)
                for cc in range(8):
                    qkt = xin.tile([128, SC], mmdt, tag="xin", name="qkt")
                    nc.sync.dma_start(out=qkt[:], in_=qkT[cc, :, ssl])
                    nc.tensor.matmul(
                        pj[:, :],
                        wqk[:, bass.ts(cc, 128)],
                        qkt[:],
                        start=(cc == 0),
                        stop=(cc == 7),
                    )
                for c in range(4):
                    vt = xin.tile([128, SC], f32, tag="xinv", name="vt")
                    nc.sync.dma_start(out=vt[:], in_=vT[bass.ts(c, 128), ssl])
                    # vp (natural layout) for the 4 t-chunks of this s-chunk:
                    # out[t, dv] accumulated over D; exact fp32. All 16 matmuls
                    # into this bank form one accumulation group — the first
                    # start marks the whole bank pending-zero, later first
                    # writes to each slice overwrite.
                    for r in range(4):
                        nc.tensor.matmul(
                            pjv[:, bass.ts(r, DV)],
                            vt[:, bass.ts(r, 128)],
                            wv[:, bass.ts(c, 64)],
                            start=(c == 0 and r == 0),
                            stop=(c == 3 and r == 3),
                        )

                nc.vector.tensor_scalar_add(qpT[sc][:], pj[0:64, :], bias[:, 0:1])
                nc.vector.tensor_scalar_add(kpT[sc][:], pj[64:128, :], bias[:, 1:2])
                for r in range(4):
                    j = 4 * sc + r
                    # vpe = [(vp + bv) * E | E]
                    nc.vector.tensor_add(
                        vpe[j][:, 0:DV], pjv[:, bass.ts(r, DV)], bvb[:]
                    )
                    nc.vector.tensor_scalar_mul(
                        vpe[j][:, 0:DV], vpe[j][:, 0:DV], E[:, j : j + 1]
                    )
                    nc.vector.tensor_copy(vpe[j][:, DV : DV + 1], E[:, j : j + 1])

                # ---- attention column sc ----
                av = ps_av.tile([128, SC], f32, tag="av")
                njt = 4 * sc + 4  # active t-chunks in this column
                for g in range(njt // 2):
                    qk = ps_qk.tile([128, 2 * SC], f32, tag="qk")
                    for r2 in range(2):
                        j = 2 * g + r2
                        nc.tensor.matmul(
                            qk[:, bass.ts(r2, SC)],
                            kpT[j // 4][:, bass.ts(j % 4, 128)],
                            qpT[sc][:],
                            start=True,
                            stop=True,
                        )
                    for r2 in range(2):
                        j = 2 * g + r2
                        if j >= 4 * sc:  # diagonal tile: additive causal mask
                            rr = j - 4 * sc
                            w_hi = 128 * (rr + 1)
                            nc.vector.tensor_add(
                                qk[:, r2 * SC : r2 * SC + w_hi],
                                qk[:, r2 * SC : r2 * SC + w_hi],
                                maskbig[:, 384 - 128 * rr : 384 - 128 * rr + w_hi],
                            )
                    P = pp.tile([128, 2 * SC], mmdt, tag="P")
                    nc.scalar.activation(P[:], qk[:], Exp, scale=0.125)
                    for r2 in range(2):
                        j = 2 * g + r2
                        nc.tensor.matmul(
                            av[0 : DV + 1, :],
                            vpe[j][:],
                            P[:, bass.ts(r2, SC)],
                            start=(j == 0),
                            stop=(j == njt - 1),
                        )

                # ---- column postprocess: transpose avT back + normalize ----
                avsb = sbw.tile([DV + 1, SC], f32, tag="avsb")
                nc.vector.tensor_copy(avsb[:], av[0 : DV + 1, :])
                for m in range(SC // 128):
                    tp = ps_av.tile([128, SC], f32, tag="av")
                    nc.tensor.transpose(
                        tp[:, 0 : DV + 1],
                        avsb[:, bass.ts(m, 128)],
                        ident[0 : DV + 1, 0 : DV + 1],
                    )
                    rcp = sbw.tile([128, 1], f32, tag="rcp")
                    nc.vector.tensor_scalar_add(rcp[:], tp[:, DV : DV + 1], 1e-10)
                    nc.vector.reciprocal(rcp[:], rcp[:])
                    ot = sbw.tile([128, DV], f32, tag="ot")
                    nc.vector.tensor_scalar_mul(ot[:], tp[:, 0:DV], rcp[:])
                    nc.sync.dma_start(
                        out=out_d[bass.ds(sc * SC + m * 128, 128), :], in_=ot[:]
                    )

    nc.compile()
    return nc


def _in_maps(inputs):
    q = np.asarray(inputs["q"], dtype=np.float32)
    k = np.asarray(inputs["k"], dtype=np.float32)
    v = np.asarray(inputs["v"], dtype=np.float32)
    pad = np.asarray(inputs["pad_masks"], dtype=np.float32)
    Wq = np.asarray(inputs["Wq"], dtype=np.float32)
    Wk = np.asarray(inputs["Wk"], dtype=np.float32)
    Wv = np.asarray(inputs["Wv"], dtype=np.float32)
    bq = np.asarray(inputs["bq"], dtype=np.float32)
    bk = np.asarray(inputs["bk"], dtype=np.float32)
    bv = np.asarray(inputs["bv"], dtype=np.float32)

    def packw(W):
        # [512, 64] -> [128, 256] with W[128c + p, m] at [p, 64c + m]
        return np.ascontiguousarray(
            W.reshape(4, 128, 64).transpose(1, 0, 2).reshape(128, 256)
        )

    # block-diagonal q/k weights: wqk[p, 128cc + m] over 64-deep D-chunks
    wqk_p = np.zeros((128, 8 * 128), np.float32)
    for cc in range(8):
        wqk_p[0:64, 128 * cc : 128 * cc + 64] = Wq[64 * cc : 64 * cc + 64, :]
        wqk_p[64:128, 128 * cc + 64 : 128 * cc + 128] = Wk[64 * cc : 64 * cc + 64, :]
    wv_p = packw(Wv)
    bias = np.ascontiguousarray(np.stack([bq, bk, bv], axis=1))

    maps = []
    for b in range(B):
        maps.append(
            {
                "qkT": np.ascontiguousarray(
                    np.concatenate(
                        [q[b].T.reshape(8, 64, S), k[b].T.reshape(8, 64, S)], axis=1
                    )
                ),
                "vT": np.ascontiguousarray(v[b].T),
                "wqk": wqk_p,
                "wv": wv_p,
                "bias": bias,
                "bvrow": np.ascontiguousarray(bv.reshape(1, DV)),
                "padT": np.ascontiguousarray(pad[b, 0].reshape(NT, 128).T),
            }
        )
    return maps


def kernel(**inputs) -> np.ndarray:
    global _prog
    if _prog is None:
        _prog = _build_program()
    from concourse.bass_utils import run_bass_kernel_spmd

    res = run_bass_kernel_spmd(
        _prog, _in_maps(inputs), core_ids=list(range(NCORES)), trace=CFG["trace"]
    )
    kernel.last_result = res
    return np.stack([res.results[i]["out"] for i in range(NCORES)], axis=0)


# revision 13
# speedup vs baseline: 1.3183x; 1.3183x over previous
"""Trainium2 Bass kernel for a single-head attention block (B=8, S=2048, D=512, dk=dv=64).

Sharding: one batch element per NeuronCore (8 cores, data parallel).

Per-core algorithm (batch b), all in "transposed" layouts chosen so that every
matmul contraction runs over the SBUF partition axis:

  host:   qT/kT/vT = q[b].T etc                           [D=512, S=2048]
  proj:   qpT[d,s] = sum_D Wq[D,d] qT[D,s] + bq[d]        [64, S]   (kpT same)
          vp[t,dv] = sum_D vT[D,t] Wv[D,dv] + bv[dv]      [S, 64]   (natural)
  scores: sT[t,s]  = sum_d kpT[d,t] qpT[d,s]              (t chunks of 128)
  P       = exp(sT * 1/8 + pad[t])  (ACT, scale+bias fused; no max-subtraction:
            scores are O(5), exp is safe in fp32 and matches the reference
            softmax exactly up to fp rounding)
  causal  = multiply P by 0/1 upper-tri mask on diagonal tiles; skip tiles
            that are fully masked
  AV:     avT[dv,s] = sum_t vpe[t,dv] P[t,s]  with vpe = [vp * E | E],
            E[t] = exp(pad[t])  (folds the pad mask into vp);  row 64 of avT
            is then the softmax denominator sum_t P~[t,s]
  out:    out[s,dv] = avT[dv,s] / (avT[64,s] + 1e-10)  (PE transpose + DVE)
"""

import numpy as np

B, S, D, DK, DV = 8, 2048, 512, 64, 64
NCORES = 8
SC = 512              # s-chunk (attention column) width
NSC = S // SC         # 4
TCH = 128             # t-chunk height
NT = S // TCH         # 16

CFG = dict(
    # float16: 1 cyc/row matmuls + fast weight load + half DMA, ~3.5e-4 rel err
    # float32r: TF32-like, ~2.9e-4, but 2-pass weight loads and 4-byte DMA
    # float32: exact, 4 cyc/row
    qk_dtype="float16",    # q/k projections + scores matmul precision
    v_dtype="float16",     # v projection, P (attention weights), AV matmul
    trace=False,           # collect NTFF profile (set by test.py)
)

_prog = None


def _build_program():
    from contextlib import ExitStack

    import concourse.bass as bass
    import concourse.mybir as mybir
    import concourse.tile as tile
    from concourse import bacc

    f32 = mybir.dt.float32
    # engines writing a reduced-precision tile round it on write, so the BIR
    # verifier accepts the matmul reads; np view of float32r is float32
    mmdt = getattr(mybir.dt, CFG["qk_dtype"])
    vdt = getattr(mybir.dt, CFG["v_dtype"])

    nc = bacc.Bacc(
        trn_type="TRN2",
        target_bir_lowering=False,
        debug=False,
        num_devices=NCORES,
    )

    # qkT[cc] = [qT[64cc:64cc+64, :]; kT[64cc:64cc+64, :]] — 64-deep D-chunks of
    # q and k interleaved so one DMA feeds the block-diagonal projection matmul
    qkT = nc.dram_tensor("qkT", [8, 128, S], mmdt, kind="ExternalInput").ap()
    vT = nc.dram_tensor("vT", [D, S], vdt, kind="ExternalInput").ap()
    # wqk[cc] = [[Wq[64cc:64cc+64, :], 0], [0, Wk[64cc:64cc+64, :]]] (block-diag)
    wqk_d = nc.dram_tensor("wqk", [128, 8 * 128], mmdt, kind="ExternalInput").ap()
    # wv packed: [p, (c, m)] with Wv[128c + p, m] at [p, 64c + m]
    wv_d = nc.dram_tensor("wv", [128, 256], vdt, kind="ExternalInput").ap()
    bias_d = nc.dram_tensor("bias", [64, 3], f32, kind="ExternalInput").ap()
    bvrow_d = nc.dram_tensor("bvrow", [1, DV], f32, kind="ExternalInput").ap()
    padT_d = nc.dram_tensor("padT", [128, NT], f32, kind="ExternalInput").ap()
    out_d = nc.dram_tensor("out", [S, DV], f32, kind="ExternalOutput").ap()

    with tile.TileContext(nc) as tc:
        with ExitStack() as ctx:
            const = ctx.enter_context(tc.tile_pool(name="const", bufs=1))
            xin = ctx.enter_context(tc.tile_pool(name="xin", bufs=8))
            pp = ctx.enter_context(tc.tile_pool(name="pp", bufs=3))
            sbw = ctx.enter_context(tc.tile_pool(name="sbw", bufs=3))
            ps_qk = ctx.enter_context(tc.tile_pool(name="ps_qk", bufs=2, space="PSUM"))
            ps_pj = ctx.enter_context(tc.tile_pool(name="ps_pj", bufs=2, space="PSUM"))
            ps_pjv = ctx.enter_context(tc.tile_pool(name="ps_pjv", bufs=1, space="PSUM"))
            ps_av = ctx.enter_context(tc.tile_pool(name="ps_av", bufs=1, space="PSUM"))

            # ---- constants ----
            wqk = const.tile([128, 8 * 128], mmdt, tag="wqk")
            nc.sync.dma_start(out=wqk[:], in_=wqk_d[:])
            wv = const.tile([128, 256], vdt, tag="wv")
            nc.sync.dma_start(out=wv[:], in_=wv_d[:])
            bias = const.tile([64, 3], f32, tag="bias")
            nc.sync.dma_start(out=bias[:], in_=bias_d[:])
            padT = const.tile([128, NT], f32, tag="padT")
            nc.sync.dma_start(out=padT[:], in_=padT_d[:])
            # bv broadcast across partitions (bv varies along the free axis of vp)
            bvb = const.tile([128, DV], f32, tag="bvb")
            nc.sync.dma_start(out=bvb[:], in_=bvrow_d.partition_broadcast(128))

            # E[t] = exp(pad[t])
            E = const.tile([128, NT], f32, tag="E")
            nc.scalar.activation(E[:], padT[:], mybir.ActivationFunctionType.Exp)

            # identity for PE transposes
            ident = const.tile([128, 128], f32, tag="ident")
            nc.gpsimd.memset(ident[:], 0.0)
            nc.gpsimd.affine_select(
                out=ident[:],
                in_=ident[:],
                compare_op=mybir.AluOpType.not_equal,
                fill=1.0,
                base=0,
                pattern=[[-1, 128]],
                channel_multiplier=1,
            )

            # shifted causal mask bank (additive): maskbig[u, x] = 0 if
            # x >= u + 384 (allowed) else -1e30 (masked). Slice for diagonal
            # tile r: mask_r[u, w] = maskbig[u, w + 384 - 128*r]
            maskbig = const.tile([128, 896], f32, tag="maskbig")
            nc.gpsimd.memset(maskbig[:], 0.0)
            nc.gpsimd.affine_select(
                out=maskbig[:],
                in_=maskbig[:],
                compare_op=mybir.AluOpType.is_ge,
                fill=-1.0e30,
                base=-384,
                pattern=[[1, 896]],
                channel_multiplier=-1,
            )

            # per-s-chunk projection outputs (SBUF resident)
            qpT = [
                const.tile([64, SC], mmdt, tag=f"qpT{i}", name=f"qpT{i}")
                for i in range(NSC)
            ]
            kpT = [
                const.tile([64, SC], mmdt, tag=f"kpT{i}", name=f"kpT{i}")
                for i in range(NSC)
            ]
            # per-t-chunk scaled V projections [vp * E | E]
            vpe = [
                const.tile([128, DV + 1], vdt, tag=f"vpe{j}", name=f"vpe{j}")
                for j in range(NT)
            ]

            Exp = mybir.ActivationFunctionType.Exp

            for sc in range(NSC):
                ssl = bass.ts(sc, SC)  # DRAM s-range of this chunk

                # ---- projections for s-chunk sc ----
                pj = ps_pj.tile([128, SC], f32, tag="pj")
                pjv = ps_pjv.tile([128, 4 * DV], f32, tag="pjv")
                # q & k projections via block-diagonal weights: out rows 0:64
                # = qpT chunk, rows 64:128 = kpT chunk, one matmul per 64-deep
                # D-chunk (full 128-wide array, single accumulation group)
                for cc in range(8):
                    qkt = xin.tile([128, SC], mmdt, tag="xin", name="qkt")
                    nc.sync.dma_start(out=qkt[:], in_=qkT[cc, :, ssl])
                    nc.tensor.matmul(
                        pj[:, :],
                        wqk[:, bass.ts(cc, 128)],
                        qkt[:],
                        start=(cc == 0),
                        stop=(cc == 7),
                    )
                for c in range(4):
                    vt = xin.tile([128, SC], vdt, tag="xinv", name="vt")
                    nc.sync.dma_start(out=vt[:], in_=vT[bass.ts(c, 128), ssl])
                    # vp (natural layout) for the 4 t-chunks of this s-chunk:
                    # out[t, dv] accumulated over D; exact fp32. All 16 matmuls
                    # into this bank form one accumulation group — the first
                    # start marks the whole bank pending-zero, later first
                    # writes to each slice overwrite.
                    for r in range(4):
                        nc.tensor.matmul(
                            pjv[:, bass.ts(r, DV)],
                            vt[:, bass.ts(r, 128)],
                            wv[:, bass.ts(c, 64)],
                            start=(c == 0 and r == 0),
                            stop=(c == 3 and r == 3),
                        )

                nc.vector.tensor_scalar_add(qpT[sc][:], pj[0:64, :], bias[:, 0:1])
                nc.vector.tensor_scalar_add(kpT[sc][:], pj[64:128, :], bias[:, 1:2])
                for r in range(4):
                    j = 4 * sc + r
                    # vpe = [(vp + bv) * E | E]
                    nc.vector.tensor_add(
                        vpe[j][:, 0:DV], pjv[:, bass.ts(r, DV)], bvb[:]
                    )
                    nc.vector.tensor_scalar_mul(
                        vpe[j][:, 0:DV], vpe[j][:, 0:DV], E[:, j : j + 1]
                    )
                    nc.vector.tensor_copy(vpe[j][:, DV : DV + 1], E[:, j : j + 1])

                # ---- attention column sc ----
                av = ps_av.tile([128, SC], f32, tag="av")
                njt = 4 * sc + 4  # active t-chunks in this column
                for g in range(njt // 2):
                    qk = ps_qk.tile([128, 2 * SC], f32, tag="qk")
                    for r2 in range(2):
                        j = 2 * g + r2
                        nc.tensor.matmul(
                            qk[:, bass.ts(r2, SC)],
                            kpT[j // 4][:, bass.ts(j % 4, 128)],
                            qpT[sc][:],
                            start=True,
                            stop=True,
                        )
                    for r2 in range(2):
                        j = 2 * g + r2
                        if j >= 4 * sc:  # diagonal tile: additive causal mask
                            rr = j - 4 * sc
                            w_hi = 128 * (rr + 1)
                            nc.vector.tensor_add(
                                qk[:, r2 * SC : r2 * SC + w_hi],
                                qk[:, r2 * SC : r2 * SC + w_hi],
                                maskbig[:, 384 - 128 * rr : 384 - 128 * rr + w_hi],
                            )
                    P = pp.tile([128, 2 * SC], vdt, tag="P")
                    nc.scalar.activation(P[:], qk[:], Exp, scale=0.125)
                    for r2 in range(2):
                        j = 2 * g + r2
                        nc.tensor.matmul(
                            av[0 : DV + 1, :],
                            vpe[j][:],
                            P[:, bass.ts(r2, SC)],
                            start=(j == 0),
                            stop=(j == njt - 1),
                        )

                # ---- column postprocess: transpose avT back + normalize ----
                avsb = sbw.tile([DV + 1, SC], f32, tag="avsb")
                nc.vector.tensor_copy(avsb[:], av[0 : DV + 1, :])
                for m in range(SC // 128):
                    tp = ps_av.tile([128, SC], f32, tag="av")
                    nc.tensor.transpose(
                        tp[:, 0 : DV + 1],
                        avsb[:, bass.ts(m, 128)],
                        ident[0 : DV + 1, 0 : DV + 1],
                    )
                    rcp = sbw.tile([128, 1], f32, tag="rcp")
                    nc.vector.tensor_scalar_add(rcp[:], tp[:, DV : DV + 1], 1e-10)
                    nc.vector.reciprocal(rcp[:], rcp[:])
                    ot = sbw.tile([128, DV], f32, tag="ot")
                    nc.vector.tensor_scalar_mul(ot[:], tp[:, 0:DV], rcp[:])
                    nc.sync.dma_start(
                        out=out_d[bass.ds(sc * SC + m * 128, 128), :], in_=ot[:]
                    )

    nc.compile()
    return nc


def _in_maps(inputs):
    import ml_dtypes

    np_of = {"bfloat16": ml_dtypes.bfloat16, "float16": np.float16}
    qk_np = np_of.get(CFG["qk_dtype"], np.float32)
    v_np = np_of.get(CFG["v_dtype"], np.float32)
    q = np.asarray(inputs["q"], dtype=np.float32)
    k = np.asarray(inputs["k"], dtype=np.float32)
    v = np.asarray(inputs["v"], dtype=np.float32)
    pad = np.asarray(inputs["pad_masks"], dtype=np.float32)
    Wq = np.asarray(inputs["Wq"], dtype=np.float32)
    Wk = np.asarray(inputs["Wk"], dtype=np.float32)
    Wv = np.asarray(inputs["Wv"], dtype=np.float32)
    bq = np.asarray(inputs["bq"], dtype=np.float32)
    bk = np.asarray(inputs["bk"], dtype=np.float32)
    bv = np.asarray(inputs["bv"], dtype=np.float32)

    def packw(W):
        # [512, 64] -> [128, 256] with W[128c + p, m] at [p, 64c + m]
        return np.ascontiguousarray(
            W.reshape(4, 128, 64).transpose(1, 0, 2).reshape(128, 256)
        )

    # block-diagonal q/k weights: wqk[p, 128cc + m] over 64-deep D-chunks
    wqk_p = np.zeros((128, 8 * 128), np.float32)
    for cc in range(8):
        wqk_p[0:64, 128 * cc : 128 * cc + 64] = Wq[64 * cc : 64 * cc + 64, :]
        wqk_p[64:128, 128 * cc + 64 : 128 * cc + 128] = Wk[64 * cc : 64 * cc + 64, :]
    wqk_p = wqk_p.astype(qk_np)
    wv_p = packw(Wv).astype(v_np)
    bias = np.ascontiguousarray(np.stack([bq, bk, bv], axis=1))

    maps = []
    for b in range(B):
        maps.append(
            {
                "qkT": np.ascontiguousarray(
                    np.concatenate(
                        [q[b].T.reshape(8, 64, S), k[b].T.reshape(8, 64, S)], axis=1
                    ).astype(qk_np)
                ),
                "vT": np.ascontiguousarray(v[b].T.astype(v_np)),
                "wqk": wqk_p,
                "wv": wv_p,
                "bias": bias,
                "bvrow": np.ascontiguousarray(bv.reshape(1, DV)),
                "padT": np.ascontiguousarray(pad[b, 0].reshape(NT, 128).T),
            }
        )
    return maps


def kernel(**inputs) -> np.ndarray:
    global _prog
    if _prog is None:
        _prog = _build_program()
    from concourse.bass_utils import run_bass_kernel_spmd

    res = run_bass_kernel_spmd(
        _prog, _in_maps(inputs), core_ids=list(range(NCORES)), trace=CFG["trace"]
    )
    kernel.last_result = res
    return np.stack([res.results[i]["out"] for i in range(NCORES)], axis=0)
